# revision 21
# baseline (speedup 1.0000x reference)
"""Trainium2 Bass kernel for nn_ExpertRouter (dense MoE routing).

Reference computation (per token t of T=4096, D=6144, MID=512, NE=16):
    h[t,n,:] = relu(xf[t] @ w1[n] + b1[n])          # [T, NE, MID]
    e[t,n]   = h[t,n] . w2[n] + b2[n]               # [T, NE]
    g[t,:]   = softmax(xf[t] @ gw + gb)             # [T, NE]
    out[t]   = sigmoid(sum_n g[t,n] * e[t,n])

Strategy: data-parallel over tokens across 8 NeuronCores (512 tokens/core,
weights replicated, no collectives). Dominant compute = 16 expert matmuls
[512,6144]@[6144,512] per core, run in fp8-e4m3 DoubleRowSwInterleave mode
(2x bf16 throughput; SwInterleave measured ~5% faster per pass than plain
DoubleRow). x, w1 (x128), gw (x128) are quantized to e4m3; descales fold
into activation scale params. h is requantized to fp8 (x16) and the
e-dot runs as 2 fp8 DoubleRow passes against fp8 w2 (x64); the net x1024
on e folds into b2 (host) and the final sigmoid's input scale.
Softmax division is deferred: out = sigmoid((sum_n expl[n]*(e_n+b2)) / sum_n expl[n]).

Measured: DMA 416 GB/s so the 50 MB/core of fp8 w1 streams behind the PE;
w1 is DMA'd at [P, KT2, P, 2] mid-chunk granularity (786 KB) on a 3-deep
prefetch ring (3 beat 4 by ~33 us in two reversed-order paired A/Bs); x is double-buffered so its DMA overlaps the previous
iteration's tail.

HW notes (measured on trn2, do not trust CoreSim for these):
- fp8 DoubleRow is 2x bf16 (cost model claims 4x); 512-col pass ~238 ns
  vs 213 ideal; SwInterleave shaves ~4-6%/pass; repeated identical
  stationaries do NOT skip Ldweights (x-stationary redesign is useless).
- walrus codegen crashes on DoubleRow/SwInterleave with narrow
  stationaries (1-wide DR, 16-wide SWI) - hence the 32-wide zero-padded
  w2 and plain-DR gating.
- deeper prefetch/PSUM rings (wbufs=6/8, ps_hbufs=4, hbufs=2/4) all
  measured slower than wbufs=4/ps_hbufs=3/hbufs=3 in paired A/B.
- timing noise: +-7% between processes, ~20 us slow drift within one;
  only adjacent paired A/B comparisons are trustworthy.
- steady-state per-rep ~430-438 us (slope, reps_hi=128); baseline bf16
  kernel was 868 us.
"""

import contextlib
import numpy as np
import ml_dtypes

# problem constants (hardcoded per harness contract)
B, NW, WS, FD = 16, 256, 8, 96
D = WS * WS * FD          # 6144
MID = 512
NE = 16
T = B * NW                # 4096 tokens
NCORES = 8
TOK = T // NCORES         # 512 tokens per core
P = 128                   # partitions
KT = D // P               # 48 contraction tiles
KT2 = KT // 2             # 24 DoubleRow k-steps (256 contraction per pass)
MT = MID // P             # 4 mid tiles
X_SCALE = 128.0           # w1/gw pre-scale: U(-1/sqrt(D),..) -> e4m3 normal range
H_SCALE = 16.0            # h pre-scale into e4m3 (h in [0,~4])
W2_SCALE = 64.0           # w2 pre-scale: U(-1/sqrt(MID),..) -> e4m3 normal range
E_SCALE = H_SCALE * W2_SCALE  # net scale on e_ps; folded into b2 + final sigmoid
XCH = 4                   # xq DMA chunks (6 k2-steps each)

_CACHE = {}


def _build(reps=1, wbufs=3, xbufs=2, no_e=False, no_gate=False, ps_hbufs=3, use_swi=True, fp8_e=True, wsplit=1, hbufs=3, db_consts=False):
    """Build + compile the per-core SPMD bass program. Returns nc.

    reps>1 wraps the whole body in a Tile For loop - used only for
    slope-based HW timing (fixed dispatch overhead cancels between rep
    counts); the graded kernel uses reps=1 (no loop)."""
    import concourse.tile as tile
    from concourse import bacc, mybir

    bf16 = mybir.dt.bfloat16
    fp8 = mybir.dt.float8e4
    f32 = mybir.dt.float32
    AF = mybir.ActivationFunctionType
    ALU = mybir.AluOpType
    SWI = (mybir.MatmulPerfMode.DoubleRowSwInterleave if use_swi
           else mybir.MatmulPerfMode.DoubleRow)
    DR = mybir.MatmulPerfMode.DoubleRow

    nc = bacc.Bacc("TRN2", target_bir_lowering=False, debug=False)

    xq_d = nc.dram_tensor("xq", [P, KT2, 2, TOK], fp8, kind="ExternalInput").ap()
    # mt-major chunks so each [P, KT2, P, 2] mid-chunk is one contiguous DMA;
    # last two dims are the SwInterleave layout (reversed mid, row-pair minor)
    w1_d = nc.dram_tensor(
        "w1", [NE, MT, P, KT2, P, 2], fp8, kind="ExternalInput"
    ).ap()
    gw_d = nc.dram_tensor("gw", [P, KT2, 2, NE], fp8, kind="ExternalInput").ap()
    b1_d = nc.dram_tensor("b1", [P, NE, MT], f32, kind="ExternalInput").ap()
    w2_d = nc.dram_tensor("w2", [P, NE, 2, 2, 32], fp8, kind="ExternalInput").ap()
    b2_d = nc.dram_tensor("b2", [1, NE], f32, kind="ExternalInput").ap()
    gb_d = nc.dram_tensor("gb", [NE, 1], f32, kind="ExternalInput").ap()
    out_d = nc.dram_tensor("out", [1, TOK], f32, kind="ExternalOutput").ap()

    with tile.TileContext(nc) as tc:
        loop_ctx = (
            tc.For_i(0, reps, 1) if reps > 1 else contextlib.nullcontext()
        )
        with (
            loop_ctx,
            tc.tile_pool(name="consts", bufs=1) as consts,
            tc.tile_pool(name="consts2", bufs=2 if db_consts else 1) as consts2,
            tc.tile_pool(name="xpool", bufs=xbufs) as xpool,
            tc.tile_pool(name="wpool", bufs=wbufs) as wpool,
            tc.tile_pool(name="hpool", bufs=hbufs) as hpool,
            tc.tile_pool(name="small", bufs=2) as small,
            tc.tile_pool(name="acc", bufs=1) as accp,
            tc.tile_pool(name="ps_h", bufs=ps_hbufs, space="PSUM") as ps_h,
            tc.tile_pool(name="ps_g", bufs=1, space="PSUM") as ps_g,
            tc.tile_pool(name="ps_e", bufs=2, space="PSUM") as ps_e,
        ):
            # resident x in fp8; chunked DMA so gating/expert-0 passes can
            # start as soon as early k2 chunks land
            xq = xpool.tile([P, KT2, 2, TOK], fp8)
            kch = KT2 // XCH
            for c in range(XCH):
                nc.sync.dma_start(
                    xq[:, c * kch:(c + 1) * kch, :, :],
                    xq_d[:, c * kch:(c + 1) * kch, :, :],
                )
            gw = consts.tile([P, KT2, 2, NE], fp8)
            nc.sync.dma_start(gw[:], gw_d[:])
            b1 = consts2.tile([P, NE, MT], f32)
            nc.sync.dma_start(b1[:], b1_d[:])
            w2 = consts2.tile([P, NE, 2, 2, 32], fp8)
            nc.sync.dma_start(w2[:], w2_d[:])
            b2 = consts2.tile([1, NE], f32)
            nc.sync.dma_start(b2[:], b2_d[:])
            gb = consts.tile([NE, 1], f32)
            nc.sync.dma_start(gb[:], gb_d[:])
            ones = consts.tile([NE, 1], f32)
            nc.vector.memset(ones[:], 1.0)

            # gating logits: gl[e, t] = sum_d 128*gw[d, e] * x[d, t] (fp8)
            expl = consts.tile([NE, TOK], f32)
            if no_gate:
                nc.vector.memset(expl[:], 1.0)
            else:
                gl = ps_g.tile([NE, TOK], f32)
                for k2 in range(KT2):
                    # plain DoubleRow: SwInterleave crashes walrus codegen
                    # for narrow (16-wide) stationaries
                    nc.tensor.matmul(
                        gl[:], gw[:, k2, :, :], xq[:, k2, :, :],
                        start=(k2 == 0), stop=(k2 == KT2 - 1), perf_mode=DR,
                    )
                # expl[e, t] = exp(gl/128 + gb)
                nc.scalar.activation(
                    expl[:], gl[:], AF.Exp, bias=gb[:], scale=1.0 / X_SCALE
                )

            # denominator: den[t] = sum_e expl[e, t]
            den = ps_g.tile([1, TOK], f32)
            nc.tensor.matmul(den[:], ones[:], expl[:], start=True, stop=True)
            rec = consts.tile([1, TOK], f32)
            nc.vector.reciprocal(rec[:], den[:])

            # flatten expl rows onto partition 0 so per-expert weighting is
            # a partition-0 elementwise op (cross-partition move via DMA).
            # Each row's DMA is emitted lazily inside the expert loop: these
            # depend on gating output, and all DMAs issue in order through
            # one sequencer queue, so emitting them here would block the w1
            # chunk prefetches queued behind them until gating completes.
            explf = consts.tile([1, NE * TOK], f32)

            # weighted-sum accumulator on partition 0 (scaled by E_SCALE)
            u = accp.tile([1, TOK], f32)
            nc.vector.memset(u[:], 0.0)

            for n in range(NE):
                e_ps = ps_e.tile([32, TOK], f32)
                for mt in range(MT):
                    w1c = wpool.tile([P, KT2, P, 2], fp8)
                    ks = KT2 // wsplit
                    for s in range(wsplit):
                        nc.sync.dma_start(
                            w1c[:, s * ks:(s + 1) * ks, :, :],
                            w1_d[n, mt, :, s * ks:(s + 1) * ks, :, :],
                        )
                    h_ps = ps_h.tile([P, TOK], f32)
                    for k2 in range(KT2):
                        nc.tensor.matmul(
                            h_ps[:], w1c[:, k2, :, :], xq[:, k2, :, :],
                            start=(k2 == 0), stop=(k2 == KT2 - 1), perf_mode=SWI,
                        )
                    # h2[:, mt%2, :] = fp8(16 * relu(h_ps/128 + b1)); b1 is
                    # pre-scaled x16 on host so bias applies after the scale
                    if mt % 2 == 0:
                        h2 = hpool.tile([P, 2, TOK], fp8)
                    nc.scalar.activation(
                        h2[:, mt % 2, :], h_ps[:], AF.Relu,
                        bias=b1[:, n, mt:mt + 1], scale=H_SCALE / X_SCALE,
                    )
                    if not no_e and mt % 2 == 1:
                        # e_ps += (16 h) . (64 w2) over this mid-chunk pair
                        if fp8_e:
                            # stationary zero-padded to 32 cols: DoubleRow
                            # with a 1-wide stationary crashes walrus codegen
                            nc.tensor.matmul(
                                e_ps[:], w2[:, n, mt // 2, :, :], h2[:, :, :],
                                start=(mt == 1), stop=(mt == MT - 1),
                                perf_mode=DR, skip_group_check=True,
                            )
                        else:
                            for c in range(2):
                                nc.tensor.matmul(
                                    e_ps[0:1, :], w2[:, n, mt // 2, c, 0:1],
                                    h2[:, c, :],
                                    start=(mt == 1 and c == 0),
                                    stop=(mt == MT - 1 and c == 1),
                                    skip_group_check=True,
                                )
                if no_e:
                    continue
                nc.sync.dma_start(
                    explf[0:1, n * TOK:(n + 1) * TOK], expl[n:n + 1, :]
                )
                # u += (e_ps + 1024*b2[n]) * expl[n]   (b2 pre-scaled x1024)
                tmp = small.tile([1, TOK], f32)
                nc.vector.scalar_tensor_tensor(
                    tmp[:], e_ps[0:1, :], b2[0:1, n:n + 1],
                    explf[0:1, n * TOK:(n + 1) * TOK],
                    ALU.add, ALU.mult,
                )
                nc.vector.tensor_add(u[:], u[:], tmp[:])

            # out = sigmoid(u / (1024 * den))
            s = small.tile([1, TOK], f32)
            nc.vector.tensor_mul(s[:], u[:], rec[:])
            o = small.tile([1, TOK], f32)
            nc.scalar.activation(o[:], s[:], AF.Sigmoid, scale=1.0 / E_SCALE)
            nc.sync.dma_start(out_d[:], o[:])

    nc.compile()
    return nc


def _prep_inputs(x, w1, b1, w2, b2, gw, gb):
    """Host-side shard + layout prep. Returns per-core in_maps."""
    fp8np = ml_dtypes.float8_e4m3

    xf = np.ascontiguousarray(np.asarray(x, np.float32)).reshape(T, D)
    # xq[core][p, k2, c, t] = xf[core*TOK + t, k2*256 + c*128 + p]
    xqp = (
        xf.reshape(NCORES, TOK, KT2, 2, P).transpose(0, 4, 2, 3, 1).astype(fp8np)
    )
    # SwInterleave stationary layout: per partition the free bytes are
    # [A_{M-1}, B_{M-1}, ..., A_0, B_0] where A/B are the two 128-row halves
    # (c=0/1) and columns (mid) are stored reversed.
    # w1p[n, mt, p, k2, j, c] = 128 * w1[n, k2*256 + c*128 + p, mt*128 + (127-j)]
    w1s = (np.asarray(w1, np.float32) * X_SCALE).reshape(NE, KT2, 2, P, MT, P)
    w1p = np.ascontiguousarray(
        w1s[..., ::-1].transpose(0, 4, 3, 1, 5, 2).astype(fp8np)
    )
    # gwp[p, k2, c, e] = 128 * gw[k2*256 + c*128 + p, e]  (plain DoubleRow)
    gws = (np.asarray(gw, np.float32) * X_SCALE).reshape(KT2, 2, P, NE)
    gwp = np.ascontiguousarray(gws.transpose(2, 0, 1, 3).astype(fp8np))
    # b1 pre-scaled x16 (applied after the H_SCALE/X_SCALE activation scale)
    b1p = np.ascontiguousarray(
        (np.asarray(b1, np.float32) * H_SCALE)
        .reshape(NE, MT, P)
        .transpose(2, 0, 1)
    )
    # w2 fp8 pairs, zero-padded to 32 stationary cols (col 0 is live):
    # w2p[p, n, mtp, c, 0] = 64 * w2[n, (2*mtp + c)*128 + p]
    w2p = np.zeros((P, NE, 2, 2, 32), fp8np)
    w2p[..., 0] = (
        (np.asarray(w2, np.float32) * W2_SCALE)
        .reshape(NE, 2, 2, P)
        .transpose(3, 0, 1, 2)
        .astype(fp8np)
    )
    b2p = np.asarray(b2, np.float32).reshape(1, NE) * E_SCALE
    gbp = np.asarray(gb, np.float32).reshape(NE, 1)

    in_maps = []
    for c in range(NCORES):
        in_maps.append(
            {
                "xq": np.ascontiguousarray(xqp[c]),
                "w1": w1p,
                "gw": gwp,
                "b1": b1p,
                "w2": w2p,
                "b2": b2p,
                "gb": gbp,
            }
        )
    return in_maps


def kernel(x, w1, b1, w2, b2, gw, gb):
    from concourse import bass_utils

    if "nc" not in _CACHE:
        _CACHE["nc"] = _build()
    nc = _CACHE["nc"]
    in_maps = _prep_inputs(x, w1, b1, w2, b2, gw, gb)
    res = bass_utils.run_bass_kernel_spmd(nc, in_maps, core_ids=list(range(NCORES)))
    out = np.concatenate([r["out"].reshape(TOK) for r in res.results])
    return out.reshape(B, NW).astype(np.float32)
